# revision 1
# baseline (speedup 1.0000x reference)
"""GAT tree-aggregation kernel, data-parallel over 8 NeuronCores.

Sharding: pure data parallel on batch B=1024 -> 128 per core; params
replicated. The per-head projection is applied AFTER the attention-
weighted sum of input features (attn @ X) @ W == attn @ (X @ W), which
cuts matmul FLOPs by (S+1)x vs the reference einsum order.
"""
import numpy as np
import jax
import jax.numpy as jnp
from functools import partial

NEG_SLOPE = 0.2
N_CORES = 8


def _gat_layer_fast(x_self, x_neigh, fc_w, a_self, a_neigh):
    # x_self: (b, P, F), x_neigh: (b, P, S, F), fc_w: (H, F, D)
    x_all = jnp.concatenate([x_self[:, :, None, :], x_neigh], axis=-2)  # (b,P,S1,F)
    logit_self = jnp.einsum('bpf,hf->bph', x_self, a_self)
    logit_all = jnp.einsum('bpsf,hf->bpsh', x_all, a_neigh)
    logits = jax.nn.leaky_relu(logit_self[:, :, None, :] + logit_all, NEG_SLOPE)
    attn = jax.nn.softmax(logits, axis=-2)                 # (b,P,S1,H)
    z = jnp.einsum('bpsh,bpsf->bphf', attn, x_all)         # weighted sum FIRST
    h = jnp.einsum('bphf,hfd->bphd', z, fc_w)              # tiny projection
    b, P, H, D = h.shape
    return h.reshape(b, P, H * D)


def _forward(x0, x1, x2, w0_fc, a0_self, a0_neigh, w1_fc, a1_self, a1_neigh, fc_w):
    b = x0.shape[0]
    h0 = _gat_layer_fast(x0, x1.reshape(b, 1, 10, -1), w0_fc, a0_self, a0_neigh)
    h1 = _gat_layer_fast(x1, x2.reshape(b, 10, 25, -1), w0_fc, a0_self, a0_neigh)
    h0 = _gat_layer_fast(h0, h1.reshape(b, 1, 10, -1), w1_fc, a1_self, a1_neigh)
    return h0[:, 0] @ fc_w


_pmapped = jax.pmap(_forward, axis_name='i',
                    in_axes=(0, 0, 0, None, None, None, None, None, None, None))


def kernel(x0, x1, x2, w0_fc, a0_self, a0_neigh, w1_fc, a1_self, a1_neigh, fc_w):
    B = x0.shape[0]
    bs = B // N_CORES

    def shard(a):
        return np.ascontiguousarray(np.asarray(a).reshape(N_CORES, bs, *a.shape[1:]))

    out = _pmapped(shard(x0), shard(x1), shard(x2),
                   jnp.asarray(w0_fc), jnp.asarray(a0_self), jnp.asarray(a0_neigh),
                   jnp.asarray(w1_fc), jnp.asarray(a1_self), jnp.asarray(a1_neigh),
                   jnp.asarray(fc_w))
    return np.asarray(out).reshape(B, -1).astype(np.float32)
